# revision 1
# baseline (speedup 1.0000x reference)
"""Trainium2 Bass kernel for nn_CMAModel (memory-augmented causal attention).

Sharding: 8 cores = 2 batches x 4 head-groups. Each core handles one batch and
4 heads (256 channels); the output projection is row-parallel and the 4
per-batch partials are summed on the host.

Per-core device program (all transposed, channels on partitions):
  qT = WqT.T @ xT, kT (incl. memory cols), V rows (S x per-head cols + ones col)
  gate logits from host-folded Wg = gate_w @ Wq; gsig = sigmoid
  per (head, T-chunk of 512): scoresT tiles [128S, 512T] -> exp (ACT, scale)
    -> causal tri-mask on diagonal blocks -> PV matmuls accumulate
    A_chunk/A_mem [65, 512] in PSUM; ones column gives row-sums (Z) for free
  combine: Y = (A_c + sig(gate)*A_m) / Z  via per-lane broadcasts (DMA)
  depthwise causal conv K=4 + residual + bias on [256ch, T]
  out partial [T, 1024] = R.T @ WoT  (PSUM -> DRAM)
"""
import contextlib
import ctypes
import os
import sys
import types

import numpy as np

# ---------------------------------------------------------------- constants
B, T, C = 2, 2048, 1024
H, HD = 16, 64
M = 256
G = 4                 # head-groups (cores per batch)
HPG = H // G          # 4 heads per core
CPG = HPG * HD        # 256 channels per core
S = T + 2 * M         # 2560 kv rows
SM = 2 * M            # 512 memory rows
NKT = C // 128        # 8 contraction tiles
NST = S // 128        # 20 S tiles (16 chunk + 4 mem)
TC = 512              # T chunk size
NTC = T // TC         # 4
SCALE = 1.0 / float(np.sqrt(HD))

_MM_DTYPE = os.environ.get("BASS_MM_DTYPE", "float32r")

_BUILT = None


# ------------------------------------------------------- axon NTFF hook shim
def _install_ntff_hook():
    """The agent image lacks antenv.axon_hooks; synthesize it so
    run_bass_kernel_spmd(trace=True) can capture NTFF profiles."""
    if "antenv.axon_hooks" in sys.modules:
        return
    so_path = "/opt/axon/libaxon_pjrt.so"
    hook = None
    if os.path.exists(so_path):
        try:
            lib = ctypes.CDLL(so_path)
            if hasattr(lib, "axon_start_nrt_profile"):
                lib.axon_start_nrt_profile.argtypes = [
                    ctypes.POINTER(ctypes.c_int64),
                    ctypes.c_size_t,
                ]
                lib.axon_start_nrt_profile.restype = ctypes.c_int64
                lib.axon_stop_nrt_profile.argtypes = [ctypes.c_char_p]
                lib.axon_stop_nrt_profile.restype = ctypes.c_int64

                @contextlib.contextmanager
                def _hook(output_dir, device_ids):
                    import jax

                    jax.devices()
                    if device_ids:
                        ids = (ctypes.c_int64 * len(device_ids))(*device_ids)
                        rc = lib.axon_start_nrt_profile(ids, len(device_ids))
                    else:
                        rc = lib.axon_start_nrt_profile(None, 0)
                    if rc != 0:
                        raise RuntimeError(f"axon_start_nrt_profile rc={rc}")
                    try:
                        yield
                    finally:
                        n = lib.axon_stop_nrt_profile(str(output_dir).encode())
                        if n < 0:
                            raise RuntimeError(f"axon_stop_nrt_profile rc={n}")

                hook = _hook
        except OSError:
            pass
    mod = types.ModuleType("antenv.axon_hooks")
    mod.get_axon_ntff_profile_hook = lambda: hook
    mod.set_axon_ntff_profile_hook = lambda h: None
    sys.modules["antenv.axon_hooks"] = mod


# ------------------------------------------------------------- device build
def _build_program():
    import concourse.tile as tile
    from concourse import bacc, mybir
    from concourse.masks import make_upper_triangular

    f32 = mybir.dt.float32
    mdt = getattr(mybir.dt, _MM_DTYPE)  # dtype of all matmul operands

    def mm(ap):
        return ap

    nc = bacc.Bacc("TRN2", target_bir_lowering=False, debug=False, num_devices=8)

    xT = nc.dram_tensor("xT", [C, T], mdt, kind="ExternalInput").ap()
    memT = nc.dram_tensor("memT", [C, SM], mdt, kind="ExternalInput").ap()
    WqT = nc.dram_tensor("WqT", [C, CPG], mdt, kind="ExternalInput").ap()
    WkT = nc.dram_tensor("WkT", [C, CPG], mdt, kind="ExternalInput").ap()
    WvTa = nc.dram_tensor("WvTa", [C, 65 * HPG], mdt, kind="ExternalInput").ap()
    WgT = nc.dram_tensor("WgT", [C, HPG], mdt, kind="ExternalInput").ap()
    gbn = nc.dram_tensor("gbn", [HPG, 1], f32, kind="ExternalInput").ap()
    WoT = nc.dram_tensor("WoT", [CPG, C], mdt, kind="ExternalInput").ap()
    K = 4
    cw = nc.dram_tensor("cw", [CPG, K], f32, kind="ExternalInput").ap()
    cb = nc.dram_tensor("cb", [CPG, 1], f32, kind="ExternalInput").ap()
    out = nc.dram_tensor("out", [T, C], f32, kind="ExternalOutput").ap()
    dbg_on = bool(int(os.environ.get("BASS_DBG", "0")))
    dbg = (
        nc.dram_tensor("dbg", [130, TC], f32, kind="ExternalOutput").ap()
        if dbg_on
        else None
    )

    Exp = mybir.ActivationFunctionType.Exp

    with tile.TileContext(nc) as tc:
        with contextlib.ExitStack() as ctx:
            const = ctx.enter_context(tc.tile_pool(name="const", bufs=1))
            xpool = ctx.enter_context(tc.tile_pool(name="xpool", bufs=2))
            sb = ctx.enter_context(tc.tile_pool(name="sb", bufs=1))
            work = ctx.enter_context(tc.tile_pool(name="work", bufs=3))
            small = ctx.enter_context(tc.tile_pool(name="small", bufs=1))
            psum = ctx.enter_context(
                tc.tile_pool(name="psum", bufs=1, space="PSUM")
            )
            drs = ctx.enter_context(tc.tile_pool(name="drs", bufs=4, space="DRAM"))

            # ---- constants / weights
            wq_s = const.tile([128, NKT, CPG], mdt)
            nc.sync.dma_start(out=wq_s, in_=WqT.rearrange("(a p) n -> p a n", p=128))
            wk_s = const.tile([128, NKT, CPG], mdt)
            nc.sync.dma_start(out=wk_s, in_=WkT.rearrange("(a p) n -> p a n", p=128))
            wva_s = const.tile([128, NKT, 65 * HPG], mdt)
            nc.sync.dma_start(out=wva_s, in_=WvTa.rearrange("(a p) n -> p a n", p=128))
            wg_s = const.tile([128, NKT, HPG], mdt)
            nc.sync.dma_start(out=wg_s, in_=WgT.rearrange("(a p) n -> p a n", p=128))
            wo_s = const.tile([128, 2, C], mdt)
            nc.sync.dma_start(out=wo_s, in_=WoT.rearrange("(a p) n -> p a n", p=128))
            cw_s = const.tile([128, 2, K], f32)
            nc.sync.dma_start(out=cw_s, in_=cw.rearrange("(a p) n -> p a n", p=128))
            cb_s = const.tile([128, 2, 1], f32)
            nc.sync.dma_start(out=cb_s, in_=cb.rearrange("(a p) n -> p a n", p=128))
            gbn_s = const.tile([HPG, 1], f32)
            nc.sync.dma_start(out=gbn_s, in_=gbn)

            tri = const.tile([128, 128], f32)
            make_upper_triangular(nc, tri, val=1.0, diag=True)

            # ---- persistent activations
            qT_s = sb.tile([128, 2, T], mdt)
            kT_s = sb.tile([128, 2, S], mdt)
            V_s = sb.tile([128, NST, 65 * HPG], mdt)  # [128, 20, 260]
            gsig = sb.tile([HPG, T], f32)

            xTr = xT.rearrange("(a p) t -> p a t", p=128)

            def proj_chunk(xh, tglob, tloc):
                """q/k/V/gate projections for T columns [tglob, tglob+512)."""
                for m in range(2):
                    pq = psum.tile([128, TC], f32, tag="ps", bufs=2)
                    for k in range(NKT):
                        nc.tensor.matmul(
                            pq,
                            mm(wq_s[:, k, m * 128:(m + 1) * 128]),
                            mm(xh[:, k, tloc:tloc + TC]),
                            start=(k == 0),
                            stop=(k == NKT - 1),
                        )
                    nc.vector.tensor_copy(qT_s[:, m, tglob:tglob + TC], pq)
                    pk = psum.tile([128, TC], f32, tag="ps", bufs=2)
                    for k in range(NKT):
                        nc.tensor.matmul(
                            pk,
                            mm(wk_s[:, k, m * 128:(m + 1) * 128]),
                            mm(xh[:, k, tloc:tloc + TC]),
                            start=(k == 0),
                            stop=(k == NKT - 1),
                        )
                    nc.vector.tensor_copy(kT_s[:, m, tglob:tglob + TC], pk)
                for mt in range(TC // 128):
                    st = tglob // 128 + mt
                    pv = psum.tile([128, 65 * HPG], f32, tag="ps", bufs=2)
                    for k in range(NKT):
                        nc.tensor.matmul(
                            pv,
                            mm(xh[:, k, tloc + mt * 128:tloc + (mt + 1) * 128]),
                            mm(wva_s[:, k, :]),
                            start=(k == 0),
                            stop=(k == NKT - 1),
                        )
                    nc.vector.tensor_copy(V_s[:, st, :], pv)
                    oc = V_s[:, st, 64:65 * HPG:65]
                    nc.vector.tensor_scalar(
                        oc, oc, 0.0, 1.0,
                        mybir.AluOpType.mult, mybir.AluOpType.add,
                    )
                pg = psum.tile([HPG, TC], f32, tag="ps", bufs=2)
                for k in range(NKT):
                    nc.tensor.matmul(
                        pg,
                        mm(wg_s[:, k, :]),
                        mm(xh[:, k, tloc:tloc + TC]),
                        start=(k == 0),
                        stop=(k == NKT - 1),
                    )
                # gsig <- exp(-(l + gate_b)) for now; finished below
                nc.scalar.activation(
                    gsig[:, tglob:tglob + TC], pg, Exp, bias=gbn_s, scale=-1.0
                )

            # T half 0
            xh0 = xpool.tile([128, NKT, T // 2], mdt, tag="xbig")
            for k in range(NKT):
                nc.sync.dma_start(out=xh0[:, k, :], in_=xTr[:, k, : T // 2])
            for ncn in range(2):
                proj_chunk(xh0, ncn * TC, ncn * TC)

            # memory projections
            mems = xpool.tile([128, NKT, SM], mdt, tag="xbig")
            nc.sync.dma_start(out=mems, in_=memT.rearrange("(a p) t -> p a t", p=128))
            for m in range(2):
                pk = psum.tile([128, SM], f32, tag="ps", bufs=2)
                for half in range(2):
                    for k in range(NKT):
                        nc.tensor.matmul(
                            pk[:, half * 256:(half + 1) * 256],
                            mm(wk_s[:, k, m * 128:(m + 1) * 128]),
                            mm(mems[:, k, half * 256:(half + 1) * 256]),
                            start=(k == 0),
                            stop=(k == NKT - 1),
                        )
                nc.vector.tensor_copy(kT_s[:, m, T:], pk)
            for mt in range(SM // 128):
                st = 16 + mt
                pv = psum.tile([128, 65 * HPG], f32, tag="ps", bufs=2)
                for k in range(NKT):
                    nc.tensor.matmul(
                        pv,
                        mm(mems[:, k, mt * 128:(mt + 1) * 128]),
                        mm(wva_s[:, k, :]),
                        start=(k == 0),
                        stop=(k == NKT - 1),
                    )
                nc.vector.tensor_copy(V_s[:, st, :], pv)
                oc = V_s[:, st, 64:65 * HPG:65]
                nc.vector.tensor_scalar(
                    oc, oc, 0.0, 1.0,
                    mybir.AluOpType.mult, mybir.AluOpType.add,
                )

            # T half 1
            xh1 = xpool.tile([128, NKT, T // 2], mdt, tag="xbig")
            for k in range(NKT):
                nc.sync.dma_start(out=xh1[:, k, :], in_=xTr[:, k, T // 2:])
            for ncn in range(2):
                proj_chunk(xh1, T // 2 + ncn * TC, ncn * TC)

            # finish sigmoid: gsig = 1 / (1 + exp(-(l+b)))
            nc.vector.tensor_scalar_add(gsig, gsig, 1.0)
            nc.vector.reciprocal(gsig, gsig)

            # ---- attention + combine; attnout[:, 0:2] = Y pairs, [:, 2:4] = conv out
            attnout = xpool.tile([128, 4, T], mdt, tag="xbig")

            def conv_pair(p):
                """depthwise causal conv + residual + bias on GpSimd
                (idle engine; keeps DVE stream free for combines)."""
                ctmp = small.tile([128, T], f32, tag="ctmp", bufs=2, name=f"ctmp{p}")
                y = attnout[:, p, :]
                R = attnout[:, 2 + p, :]
                nc.vector.tensor_scalar_add(R, y, cb_s[:, p, :])
                for k in range(K):
                    sh = K - 1 - k
                    if sh == 0:
                        nc.vector.tensor_scalar_mul(ctmp, y, cw_s[:, p, k:k + 1])
                        nc.vector.tensor_add(R, R, ctmp)
                    else:
                        nc.vector.tensor_scalar_mul(
                            ctmp[:, sh:], y[:, :T - sh], cw_s[:, p, k:k + 1]
                        )
                        nc.vector.tensor_add(R[:, sh:], R[:, sh:], ctmp[:, sh:])

            for hl in range(HPG):
                mq, par = divmod(hl, 2)
                ro = 64 * par
                vc = 65 * hl
                for j in range(NTC):
                    nct = 4 * (j + 1)   # visible chunk S-tiles
                    Ac = psum.tile([128, TC], f32, tag="pa", bufs=6)
                    Am = psum.tile([128, TC], f32, tag="pa", bufs=6)
                    for i in range(nct + 4):
                        is_mem = i >= nct
                        si = (16 + i - nct) if is_mem else i
                        off = 0
                        if not is_mem and si >= 4 * j:
                            off = 128 * si - TC * j
                        n = TC - off
                        ps = psum.tile([128, TC], f32, tag="ps", bufs=2)
                        nc.tensor.matmul(
                            ps[:, off:],
                            mm(kT_s[ro:ro + 64, mq, si * 128:(si + 1) * 128]),
                            mm(qT_s[ro:ro + 64, mq, TC * j + off:TC * (j + 1)]),
                            start=True,
                            stop=True,
                        )
                        Pt = work.tile([128, TC], mdt, tag="P")
                        nc.scalar.activation(
                            Pt[:, off:], ps[:, off:], Exp, scale=SCALE
                        )
                        if not is_mem and si >= 4 * j:
                            nc.vector.tensor_mul(
                                Pt[:, off:off + 128], Pt[:, off:off + 128], tri
                            )
                        dst = Am if is_mem else Ac
                        first = (i == 0) or (is_mem and i == nct)
                        last = (i == nct - 1) or (i == nct + 3)
                        nc.tensor.matmul(
                            dst[0:65, off:],
                            mm(V_s[:, si, vc:vc + 65]),
                            mm(Pt[:, off:]),
                            start=first,
                            stop=last,
                        )
                    if dbg is not None and hl == 0 and j == 0:
                        dbt = small.tile([65, 2, TC], f32, tag="dbt", bufs=1)
                        nc.vector.tensor_copy(dbt[:, 0, :], Ac[0:65, :])
                        nc.vector.tensor_copy(dbt[:, 1, :], Am[0:65, :])
                        nc.sync.dma_start(out=dbg[0:65, :], in_=dbt[:, 0, :])
                        nc.sync.dma_start(out=dbg[65:130, :], in_=dbt[:, 1, :])
                    # combine: Y = (Ac + g*Am) / Z on lanes 0..63, Z at lane 64
                    zu = small.tile([128, TC], f32, tag="zu", bufs=2)
                    nc.vector.tensor_copy(zu[64:65, :], Ac[64:65, :])
                    nc.vector.tensor_add(
                        zu[64:65, :], zu[64:65, :], Am[64:65, :]
                    )
                    # reshape Z row and gate row onto 128 lanes so reciprocal
                    # runs 128-wide (4 elems/lane) instead of 512 on one lane
                    zrg = small.tile([128, 2, TC // 128], f32, tag="zrg", bufs=2)
                    nc.sync.dma_start(out=zrg[:, 0, :], in_=zu[64:65, :])
                    nc.sync.dma_start(
                        out=zrg[:, 1, :], in_=gsig[hl:hl + 1, TC * j:TC * (j + 1)]
                    )
                    nc.vector.reciprocal(zrg[:, 0, :], zrg[:, 0, :])
                    nc.vector.tensor_mul(zrg[:, 1, :], zrg[:, 1, :], zrg[:, 0, :])
                    # bounce through DRAM so the rows can be partition-broadcast
                    zsc = drs.tile([2, TC], f32, tag="zsc", bufs=4)
                    nc.sync.dma_start(out=zsc[0:1, :], in_=zrg[:, 0, :])
                    nc.sync.dma_start(out=zsc[1:2, :], in_=zrg[:, 1, :])
                    rzb = small.tile([64, TC], f32, tag="bc", bufs=4)
                    nc.sync.dma_start(
                        out=rzb, in_=zsc[0:1, :].partition_broadcast(64)
                    )
                    gzb = small.tile([64, TC], f32, tag="bc", bufs=4)
                    nc.sync.dma_start(
                        out=gzb, in_=zsc[1:2, :].partition_broadcast(64)
                    )
                    tmp = small.tile([64, TC], f32, tag="tmp", bufs=2)
                    tmp2 = small.tile([64, TC], mdt, tag="tmp", bufs=2)
                    nc.vector.tensor_mul(tmp, Am[0:64, :], gzb)   # Am * g/Z
                    nc.vector.tensor_mul(tmp2, Ac[0:64, :], rzb)  # Ac / Z
                    if par == 0:
                        nc.vector.tensor_add(
                            attnout[0:64, mq, TC * j:TC * (j + 1)], tmp, tmp2
                        )
                    else:
                        nc.vector.tensor_add(tmp2, tmp, tmp2)
                        nc.sync.dma_start(
                            out=attnout[64:128, mq, TC * j:TC * (j + 1)], in_=tmp2
                        )
                if hl == 1 or hl == 3:
                    conv_pair(hl // 2)

            # ---- output projection: out[T, C] partial
            for mt in range(T // 128):
                for nb in range(2):
                    po = psum.tile([128, TC], f32, tag="ps", bufs=2)
                    for p in range(2):
                        nc.tensor.matmul(
                            po,
                            mm(attnout[:, 2 + p, mt * 128:(mt + 1) * 128]),
                            mm(wo_s[:, p, nb * TC:(nb + 1) * TC]),
                            start=(p == 0),
                            stop=(p == 1),
                        )
                    ot = work.tile([128, TC], f32, tag="ot", bufs=3)
                    nc.any.tensor_copy(ot, po)
                    nc.sync.dma_start(
                        out=out[mt * 128:(mt + 1) * 128, nb * TC:(nb + 1) * TC],
                        in_=ot,
                    )

    nc.compile()
    return nc


def _get_program():
    global _BUILT
    if _BUILT is None:
        _install_ntff_hook()
        _BUILT = _build_program()
    return _BUILT


# --------------------------------------------------------------- host side
def _tf32_round(a):
    """Cast to the matmul-operand dtype: TF32-round for float32r (data stays
    fp32 bits), bfloat16 for bf16 mode, passthrough for float32."""
    if _MM_DTYPE == "bfloat16":
        import ml_dtypes

        return np.ascontiguousarray(a, np.float32).astype(ml_dtypes.bfloat16)
    if _MM_DTYPE != "float32r":
        return np.ascontiguousarray(a, np.float32)
    u = np.ascontiguousarray(a, np.float32).view(np.uint32).astype(np.uint64)
    u = (u + 0x0FFF + ((u >> 13) & 1)) & np.uint64(0xFFFFE000)
    return u.astype(np.uint32).view(np.float32)


def host_prep(inputs):
    x = np.ascontiguousarray(np.asarray(inputs["x"], np.float32))
    fwd = np.asarray(inputs["fwd_mem"], np.float32)
    rev = np.asarray(inputs["rev_mem"], np.float32)
    Wq = np.asarray(inputs["Wq"], np.float32)
    Wk = np.asarray(inputs["Wk"], np.float32)
    Wv = np.asarray(inputs["Wv"], np.float32)
    Wo = np.asarray(inputs["Wo"], np.float32)
    gate_w = np.asarray(inputs["gate_w"], np.float32)
    gate_b = np.asarray(inputs["gate_b"], np.float32)
    canon_w = np.asarray(inputs["canon_w"], np.float32)
    canon_bias = np.asarray(inputs["canon_bias"], np.float32)

    Wg = (gate_w.astype(np.float64) @ Wq.astype(np.float64)).astype(np.float32)

    per_b, per_g = [], []
    for b in range(B):
        per_b.append({
            "xT": _tf32_round(x[b].T),
            "memT": _tf32_round(np.concatenate([fwd[b], rev[b]], axis=0).T),
        })
    for g in range(G):
        cs = slice(g * CPG, (g + 1) * CPG)
        WvTa = np.zeros((C, 65 * HPG), np.float32)
        for h in range(HPG):
            rows = Wv[g * CPG + h * HD: g * CPG + (h + 1) * HD]
            WvTa[:, 65 * h:65 * h + 64] = rows.T
        hs = slice(g * HPG, (g + 1) * HPG)
        per_g.append({
            "WqT": _tf32_round(Wq[cs].T),
            "WkT": _tf32_round(Wk[cs].T),
            "WvTa": _tf32_round(WvTa),
            "WgT": _tf32_round(Wg[hs].T),
            "gbn": np.ascontiguousarray(-gate_b[hs]).reshape(HPG, 1),
            "WoT": _tf32_round(Wo[:, cs].T),
            "cw": np.ascontiguousarray(canon_w[cs, 0, :]),
            "cb": np.ascontiguousarray(canon_bias[cs]).reshape(CPG, 1),
        })
    return per_b, per_g


LAST_EXEC_NS = None
LAST_RESULTS = None


def kernel(**inputs):
    global LAST_EXEC_NS, LAST_RESULTS
    from concourse.bass_utils import run_bass_kernel_spmd

    nc = _get_program()
    per_b, per_g = host_prep(inputs)
    in_maps = []
    for core in range(8):
        b, g = divmod(core, G)
        m = {}
        m.update(per_b[b])
        m.update(per_g[g])
        in_maps.append(m)

    trace = bool(int(os.environ.get("KERNEL_TRACE", "0")))
    kw = {}
    if trace:
        tcores = os.environ.get("KERNEL_TRACE_CORES", "0")
        kw = dict(
            trace=True,
            trace_cores=[int(c) for c in tcores.split(",")],
            tmpdir=os.environ.get("KERNEL_TRACE_DIR", None),
        )
    res = run_bass_kernel_spmd(nc, in_maps, core_ids=list(range(8)), **kw)
    LAST_EXEC_NS = res.exec_time_ns
    LAST_RESULTS = res
    outp = np.zeros((B, T, C), np.float32)
    for core in range(8):
        b = core // G
        outp[b] += res.results[core]["out"]
    return outp



# revision 6
# speedup vs baseline: 1.2769x; 1.2769x over previous
"""Trainium2 Bass kernel for nn_CMAModel (memory-augmented causal attention).

Sharding: 8 cores = 2 batches x 4 head-groups. Each core handles one batch and
4 heads (256 channels); the output projection is row-parallel and the 4
per-batch partials are summed on the host (bf16 partials).

Per-core device program (channels on partitions, bf16 matmul operands):
  proj: qT/kT (paired PSUM banks), V rows (with ones col for row-sums),
        gate logits -> tanh (same ACT table set as Exp; sigmoid = .5*tanh+.5)
  attention per (head-pair mq, T-chunk j): for each 128-row S-tile,
        scoresT for heads A,B as two K=64 row-tiled matmuls into a 2-bank
        PSUM pair -> ONE Exp activation over both banks -> causal tri-mask
        on diagonal tiles -> PV matmuls accumulate Ac/Am [65,512] per head
        (ones col gives row-sums Z for free)
  combine: Y = (Ac + g*Am)/Z. g and 1/Z are partition-broadcast with rank-1
        matmuls on the PE (lhsT=indicator const, rhs=rows of gzt), so no
        DRAM round trips. Z-recip runs 128-wide via small SBUF reshape DMAs.
  conv + out-proj interleaved per T-chunk: depthwise causal conv K=4 +
        residual + bias, then out partial [512,1024] -> DRAM (bf16).
"""
import contextlib
import ctypes
import os
import sys
import types

import numpy as np

# ---------------------------------------------------------------- constants
B, T, C = 2, 2048, 1024
H, HD = 16, 64
M = 256
G = 4                 # head-groups (cores per batch)
HPG = H // G          # 4 heads per core
CPG = HPG * HD        # 256 channels per core
S = T + 2 * M         # 2560 kv rows
SM = 2 * M            # 512 memory rows
NKT = C // 128        # 8 contraction tiles
NST = S // 128        # 20 S tiles (16 chunk + 4 mem)
TC = 512              # T chunk size
NTC = T // TC         # 4
K = 4                 # conv taps
SCALE = 1.0 / float(np.sqrt(HD))

_BUILT = None


# ------------------------------------------------------- axon NTFF hook shim
def _install_ntff_hook():
    """The agent image lacks antenv.axon_hooks; synthesize it so
    run_bass_kernel_spmd(trace=True) can capture NTFF profiles."""
    if "antenv.axon_hooks" in sys.modules:
        return
    so_path = "/opt/axon/libaxon_pjrt.so"
    hook = None
    if os.path.exists(so_path):
        try:
            lib = ctypes.CDLL(so_path)
            if hasattr(lib, "axon_start_nrt_profile"):
                lib.axon_start_nrt_profile.argtypes = [
                    ctypes.POINTER(ctypes.c_int64),
                    ctypes.c_size_t,
                ]
                lib.axon_start_nrt_profile.restype = ctypes.c_int64
                lib.axon_stop_nrt_profile.argtypes = [ctypes.c_char_p]
                lib.axon_stop_nrt_profile.restype = ctypes.c_int64

                @contextlib.contextmanager
                def _hook(output_dir, device_ids):
                    import jax

                    jax.devices()
                    if device_ids:
                        ids = (ctypes.c_int64 * len(device_ids))(*device_ids)
                        rc = lib.axon_start_nrt_profile(ids, len(device_ids))
                    else:
                        rc = lib.axon_start_nrt_profile(None, 0)
                    if rc != 0:
                        raise RuntimeError(f"axon_start_nrt_profile rc={rc}")
                    try:
                        yield
                    finally:
                        n = lib.axon_stop_nrt_profile(str(output_dir).encode())
                        if n < 0:
                            raise RuntimeError(f"axon_stop_nrt_profile rc={n}")

                hook = _hook
        except OSError:
            pass
    mod = types.ModuleType("antenv.axon_hooks")
    mod.get_axon_ntff_profile_hook = lambda: hook
    mod.set_axon_ntff_profile_hook = lambda h: None
    sys.modules["antenv.axon_hooks"] = mod


# ------------------------------------------------------------- device build
def _build_program():
    import concourse.tile as tile
    from concourse import bacc, mybir
    from concourse.masks import make_upper_triangular

    f32 = mybir.dt.float32
    mdt = mybir.dt.bfloat16

    nc = bacc.Bacc("TRN2", target_bir_lowering=False, debug=False, num_devices=8)

    xT = nc.dram_tensor("xT", [C, T], mdt, kind="ExternalInput").ap()
    memT = nc.dram_tensor("memT", [C, SM], mdt, kind="ExternalInput").ap()
    WqT = nc.dram_tensor("WqT", [C, CPG], mdt, kind="ExternalInput").ap()
    WkT = nc.dram_tensor("WkT", [C, CPG], mdt, kind="ExternalInput").ap()
    WvTa = nc.dram_tensor("WvTa", [C, 65 * HPG], mdt, kind="ExternalInput").ap()
    WgT = nc.dram_tensor("WgT", [C, HPG], mdt, kind="ExternalInput").ap()
    gb2 = nc.dram_tensor("gb2", [HPG, 1], f32, kind="ExternalInput").ap()
    WoT = nc.dram_tensor("WoT", [CPG, C], mdt, kind="ExternalInput").ap()
    ind = nc.dram_tensor("ind", [128, 256], mdt, kind="ExternalInput").ap()
    cw = nc.dram_tensor("cw", [CPG, K], f32, kind="ExternalInput").ap()
    cb = nc.dram_tensor("cb", [CPG, 1], f32, kind="ExternalInput").ap()
    out = nc.dram_tensor("out", [T, C], mdt, kind="ExternalOutput").ap()

    Exp = mybir.ActivationFunctionType.Exp
    Tanh = mybir.ActivationFunctionType.Tanh

    with tile.TileContext(nc) as tc:
        with contextlib.ExitStack() as ctx:
            const = ctx.enter_context(tc.tile_pool(name="const", bufs=1))
            xpool = ctx.enter_context(tc.tile_pool(name="xpool", bufs=2))
            sb = ctx.enter_context(tc.tile_pool(name="sb", bufs=1))
            work = ctx.enter_context(tc.tile_pool(name="work", bufs=4))
            small = ctx.enter_context(tc.tile_pool(name="small", bufs=1))
            psum = ctx.enter_context(
                tc.tile_pool(name="psum", bufs=1, space="PSUM")
            )

            # ---- constants / weights
            wq_s = const.tile([128, NKT, CPG], mdt)
            nc.sync.dma_start(out=wq_s, in_=WqT.rearrange("(a p) n -> p a n", p=128))
            wk_s = const.tile([128, NKT, CPG], mdt)
            nc.sync.dma_start(out=wk_s, in_=WkT.rearrange("(a p) n -> p a n", p=128))
            wva_s = const.tile([128, NKT, 65 * HPG], mdt)
            nc.sync.dma_start(out=wva_s, in_=WvTa.rearrange("(a p) n -> p a n", p=128))
            wg_s = const.tile([128, NKT, HPG], mdt)
            nc.sync.dma_start(out=wg_s, in_=WgT.rearrange("(a p) n -> p a n", p=128))
            wo_s = const.tile([128, 2, C], mdt)
            nc.sync.dma_start(out=wo_s, in_=WoT.rearrange("(a p) n -> p a n", p=128))
            cw_s = const.tile([128, 2, K], f32)
            nc.sync.dma_start(out=cw_s, in_=cw.rearrange("(a p) n -> p a n", p=128))
            cb_s = const.tile([128, 2, 1], f32)
            nc.sync.dma_start(out=cb_s, in_=cb.rearrange("(a p) n -> p a n", p=128))
            gb2_s = const.tile([HPG, 1], f32)
            nc.sync.dma_start(out=gb2_s, in_=gb2)
            ind_s = const.tile([128, 256], mdt)
            nc.sync.dma_start(out=ind_s, in_=ind)

            trif = const.tile([128, 128], f32)
            make_upper_triangular(nc, trif, val=1.0, diag=True)
            tri2 = const.tile([128, 2, 128], mdt)
            nc.vector.tensor_copy(tri2[:, 0, :], trif)
            nc.vector.tensor_copy(tri2[:, 1, :], trif)

            # ---- persistent activations
            # qkT_s[:, m, 0, t] = qT, [:, m, 1, t] = kT  (m = channel half)
            qkT_s = sb.tile([128, 2, 2, T], mdt)
            kTm_s = sb.tile([128, 2, SM], mdt)
            V_s = sb.tile([128, NST, 65 * HPG], mdt)
            # gzt rows (base b=64*mq): b+0 tanh_A, b+1 tanh_B, b+2 ones,
            # b+3 recipZ_A, b+4 recipZ_B
            gzt = sb.tile([128, T], mdt)
            # attnout[:, mq, 0, t] = Y, [:, mq, 1, t] = conv result
            attnout = sb.tile([128, 2, 2, T], mdt)

            # one-time inits: ones col in V, gzt ones + recip rows (rows 0-1 /
            # 64-65 are overwritten by the gate tanh per chunk; engine ops
            # need 32-aligned partition starts so memset the whole block)
            oc = V_s[:, :, 64:65 * HPG:65]
            nc.vector.memset(oc, 1.0)
            for base in (0, 64):
                nc.vector.memset(gzt[base:base + 5, :], 1.0)

            xTr = xT.rearrange("(a p) t -> p a t", p=128)

            def proj_chunk(xh, tglob, tloc):
                """q/k/V/gate projections for T columns [tglob, tglob+512)."""
                for m in range(2):
                    qk = psum.tile([128, 2, TC], f32, tag="pp", bufs=2,
                                   name=f"qk{tglob}_{m}")
                    for w, ws in ((0, wq_s), (1, wk_s)):
                        for k in range(NKT):
                            nc.tensor.matmul(
                                qk[:, w, :],
                                ws[:, k, m * 128:(m + 1) * 128],
                                xh[:, k, tloc:tloc + TC],
                                start=(k == 0),
                                stop=(k == NKT - 1),
                            )
                    nc.vector.tensor_copy(
                        qkT_s[:, m, :, tglob:tglob + TC], qk
                    )
                for mt in range(TC // 128):
                    st = tglob // 128 + mt
                    pv = psum.tile([128, 65 * HPG], f32, tag="pp", bufs=2,
                                   name=f"pv{st}")
                    for k in range(NKT):
                        nc.tensor.matmul(
                            pv,
                            xh[:, k, tloc + mt * 128:tloc + (mt + 1) * 128],
                            wva_s[:, k, :],
                            start=(k == 0),
                            stop=(k == NKT - 1),
                        )
                    # copy the 4 x 64 v-blocks, skipping the ones columns
                    nc.vector.tensor_copy(
                        V_s[:, st, :].rearrange("p (h c) -> p h c", c=65)[:, :, 0:64],
                        pv.rearrange("p (h c) -> p h c", c=65)[:, :, 0:64],
                    )
                pg = psum.tile([HPG, TC], f32, tag="pp", bufs=2,
                               name=f"pg{tglob}")
                for k in range(NKT):
                    nc.tensor.matmul(
                        pg,
                        wg_s[:, k, :],
                        xh[:, k, tloc:tloc + TC],
                        start=(k == 0),
                        stop=(k == NKT - 1),
                    )
                # sigmoid(l+b) = .5*tanh((l+b)/2) + .5; the .5 affine folds
                # into the gbc broadcast matmul via the ind/ones-row coeffs
                gtmp = small.tile([HPG, TC], mdt, tag="gt", bufs=2,
                                  name=f"gt{tglob}")
                nc.scalar.activation(
                    gtmp, pg, Tanh, bias=gb2_s, scale=0.5,
                )
                nc.sync.dma_start(
                    out=gzt[0:2, tglob:tglob + TC], in_=gtmp[0:2, :]
                )
                nc.sync.dma_start(
                    out=gzt[64:66, tglob:tglob + TC], in_=gtmp[2:4, :]
                )

            def proj_mem(mems):
                mk = psum.tile([128, 2, SM], f32, tag="pp", bufs=2, name="mk")
                for m in range(2):
                    for k in range(NKT):
                        nc.tensor.matmul(
                            mk[:, m, :],
                            wk_s[:, k, m * 128:(m + 1) * 128],
                            mems[:, k, :],
                            start=(k == 0),
                            stop=(k == NKT - 1),
                        )
                nc.vector.tensor_copy(kTm_s, mk)
                for mt in range(SM // 128):
                    st = 16 + mt
                    pv = psum.tile([128, 65 * HPG], f32, tag="pp", bufs=2,
                                   name=f"pvm{mt}")
                    for k in range(NKT):
                        nc.tensor.matmul(
                            pv,
                            mems[:, k, mt * 128:(mt + 1) * 128],
                            wva_s[:, k, :],
                            start=(k == 0),
                            stop=(k == NKT - 1),
                        )
                    nc.vector.tensor_copy(
                        V_s[:, st, :].rearrange("p (h c) -> p h c", c=65)[:, :, 0:64],
                        pv.rearrange("p (h c) -> p h c", c=65)[:, :, 0:64],
                    )

            def attn_block(mq, j):
                """Attention for head pair (2mq, 2mq+1), T chunk j."""
                base = 64 * mq
                hA, hB = 2 * mq, 2 * mq + 1
                nct = 4 * (j + 1)
                js = TC * j
                AcAm_A = psum.tile([128, 2, TC], f32, tag="pa", bufs=2,
                                   name=f"aa{mq}_{j}")
                AcAm_B = psum.tile([128, 2, TC], f32, tag="pa", bufs=2,
                                   name=f"ab{mq}_{j}")
                for i in range(nct + 4):
                    is_mem = i >= nct
                    si = (16 + i - nct) if is_mem else i
                    off = 0
                    if not is_mem and si >= 4 * j:
                        off = 128 * si - TC * j
                    sp = psum.tile([128, 2, TC], f32, tag="pp", bufs=2,
                                   name=f"sp{mq}_{j}_{i}")
                    for b, ro in ((0, 0), (1, 64)):
                        kt = (
                            qkT_s[ro:ro + 64, mq, 1, si * 128:(si + 1) * 128]
                            if si < 16
                            else kTm_s[ro:ro + 64, mq,
                                       (si - 16) * 128:(si - 15) * 128]
                        )
                        nc.tensor.matmul(
                            sp[:, b, off:],
                            kt,
                            qkT_s[ro:ro + 64, mq, 0, js + off:js + TC],
                            start=True,
                            stop=True,
                        )
                    Pt = work.tile([128, 2, TC], mdt, tag="P")
                    nc.scalar.activation(
                        Pt[:, :, off:], sp[:, :, off:], Exp, scale=SCALE
                    )
                    if not is_mem and si >= 4 * j:
                        nc.vector.tensor_mul(
                            Pt[:, :, off:off + 128], Pt[:, :, off:off + 128],
                            tri2,
                        )
                    cm = 1 if is_mem else 0
                    first = (i == 0) or (i == nct)
                    last = (i == nct - 1) or (i == nct + 3)
                    nc.tensor.matmul(
                        AcAm_A[0:65, cm, off:],
                        V_s[:, si, 65 * hA:65 * hA + 65],
                        Pt[:, 0, off:],
                        start=first,
                        stop=last,
                    )
                    nc.tensor.matmul(
                        AcAm_B[0:65, cm, off:],
                        V_s[:, si, 65 * hB:65 * hB + 65],
                        Pt[:, 1, off:],
                        start=first,
                        stop=last,
                    )
                # Z rows -> 128-wide reciprocal -> gzt recip rows (TT may
                # read at most one PSUM operand, so copy then add)
                zt = small.tile([128, 2, TC], f32, tag="zt", bufs=2,
                                name=f"zt{mq}_{j}")
                for b, AcAm in ((0, AcAm_A), (1, AcAm_B)):
                    nc.vector.tensor_copy(zt[64:65, b, :], AcAm[64:65, 0, :])
                    nc.vector.tensor_add(
                        zt[64:65, b, :], zt[64:65, b, :], AcAm[64:65, 1, :]
                    )
                zrf = small.tile([128, 8], f32, tag="zrf", bufs=2,
                                 name=f"zrf{mq}_{j}")
                nc.sync.dma_start(out=zrf, in_=zt[64:65, :, :])
                zrg = small.tile([128, 8], f32, tag="zrg", bufs=2,
                                 name=f"zrg{mq}_{j}")
                nc.vector.reciprocal(zrg, zrf)
                zrb = small.tile([128, 8], mdt, tag="zrb", bufs=2,
                                 name=f"zrb{mq}_{j}")
                nc.vector.tensor_copy(zrb, zrg)
                nc.sync.dma_start(
                    out=gzt[base + 3:base + 4, js:js + TC], in_=zrb[0:64, :]
                )
                nc.sync.dma_start(
                    out=gzt[base + 4:base + 5, js:js + TC], in_=zrb[64:128, :]
                )
                # combine per head: Y = (Ac + g*Am) * recipZ
                for hb, AcAm in ((0, AcAm_A), (1, AcAm_B)):
                    bc = psum.tile([128, 2, TC], f32, tag="pp", bufs=2,
                                   name=f"bc{mq}_{j}_{hb}")
                    co = 128 * hb
                    nc.tensor.matmul(
                        bc[0:64, 0, :],
                        ind_s[base:base + 5, co:co + 64],
                        gzt[base:base + 5, js:js + TC],
                        start=True,
                        stop=True,
                    )
                    nc.tensor.matmul(
                        bc[0:64, 1, :],
                        ind_s[base:base + 5, co + 64:co + 128],
                        gzt[base:base + 5, js:js + TC],
                        start=True,
                        stop=True,
                    )
                    # stage broadcasts in SBUF (TT allows one PSUM operand);
                    # bf16 makes the final SBUF-only multiply a 2x DVE op
                    bcS = small.tile([64, 2, TC], mdt, tag="bcS", bufs=3,
                                     name=f"bs{mq}_{j}_{hb}")
                    if hb == 0:
                        nc.scalar.copy(bcS, bc[0:64, :, :])
                    else:
                        nc.vector.tensor_copy(bcS, bc[0:64, :, :])
                    uY = small.tile([64, TC], mdt, tag="uY", bufs=3,
                                    name=f"uY{mq}_{j}_{hb}")
                    nc.vector.tensor_mul(uY, AcAm[0:64, 1, :], bcS[:, 0, :])
                    nc.vector.tensor_add(uY, uY, AcAm[0:64, 0, :])
                    if hb == 0:
                        nc.vector.tensor_mul(
                            attnout[0:64, mq, 0, js:js + TC], uY,
                            bcS[:, 1, :],
                        )
                    else:
                        ybt = small.tile([64, TC], mdt, tag="ybt", bufs=2,
                                         name=f"yb{mq}_{j}")
                        nc.vector.tensor_mul(ybt, uY, bcS[:, 1, :])
                        nc.sync.dma_start(
                            out=attnout[64:128, mq, 0, js:js + TC], in_=ybt
                        )

            def conv_chunk(j, mq):
                """depthwise causal conv + residual + bias on chunk j."""
                js, je = TC * j, TC * (j + 1)
                y = attnout[:, mq, 0, :]
                R = attnout[:, mq, 1, :]
                nc.vector.tensor_scalar_add(
                    R[:, js:je], y[:, js:je], cb_s[:, mq, :]
                )
                ctmp = small.tile([128, TC], mdt, tag="ctmp", bufs=2,
                                  name=f"ct{j}_{mq}")
                for k in range(K):
                    sh = K - 1 - k
                    if sh == 0:
                        nc.vector.tensor_scalar_mul(
                            ctmp, y[:, js:je], cw_s[:, mq, k:k + 1]
                        )
                        nc.vector.tensor_add(R[:, js:je], R[:, js:je], ctmp)
                    else:
                        a = sh if j == 0 else 0
                        nc.vector.tensor_scalar_mul(
                            ctmp[:, a:], y[:, js + a - sh:je - sh],
                            cw_s[:, mq, k:k + 1],
                        )
                        nc.vector.tensor_add(
                            R[:, js + a:je], R[:, js + a:je], ctmp[:, a:]
                        )

            def outproj_chunk(j):
                for mt in range(TC // 128):
                    row = j * 4 + mt
                    po = psum.tile([128, 2, TC], f32, tag="pp", bufs=2,
                                   name=f"po{row}")
                    for nb in range(2):
                        for p in range(2):
                            nc.tensor.matmul(
                                po[:, nb, :],
                                attnout[:, p, 1, row * 128:(row + 1) * 128],
                                wo_s[:, p, nb * TC:(nb + 1) * TC],
                                start=(p == 0),
                                stop=(p == 1),
                            )
                    ot = small.tile([128, 2, TC], mdt, tag="ot", bufs=3,
                                    name=f"ot{row}")
                    if mt % 2 == 0:
                        nc.vector.tensor_copy(ot, po)
                    else:
                        nc.scalar.copy(ot, po)
                    nc.sync.dma_start(
                        out=out[row * 128:(row + 1) * 128, :].rearrange(
                            "p (a n) -> p a n", a=2
                        ),
                        in_=ot,
                    )

            # ---- emission: proj c0, mem, c1, then attn j interleaved with
            # remaining proj chunks so PE always has dense work and ACT/DVE
            # overlap.
            xh0 = xpool.tile([128, NKT, T // 2], mdt, tag="xbig", name="xh0")
            for k in range(NKT):
                nc.sync.dma_start(out=xh0[:, k, :], in_=xTr[:, k, :T // 2])
            mems = xpool.tile([128, NKT, SM], mdt, tag="xmem", name="xmem")
            nc.sync.dma_start(out=mems, in_=memT.rearrange("(a p) t -> p a t", p=128))

            proj_chunk(xh0, 0, 0)
            proj_mem(mems)
            proj_chunk(xh0, TC, TC)

            xh1 = xpool.tile([128, NKT, T // 2], mdt, tag="xbig", name="xh1")
            for k in range(NKT):
                nc.sync.dma_start(out=xh1[:, k, :], in_=xTr[:, k, T // 2:])

            for j in range(NTC):
                if j == 2:
                    proj_chunk(xh1, 2 * TC, 0)
                elif j == 3:
                    proj_chunk(xh1, 3 * TC, TC)
                for mq in range(2):
                    attn_block(mq, j)
                for mq in range(2):
                    conv_chunk(j, mq)
                outproj_chunk(j)

    nc.compile()
    return nc


def _get_program():
    global _BUILT
    if _BUILT is None:
        _install_ntff_hook()
        _BUILT = _build_program()
    return _BUILT


# --------------------------------------------------------------- host side
def _b16(a):
    import ml_dtypes

    return np.ascontiguousarray(a, np.float32).astype(ml_dtypes.bfloat16)


def host_prep(inputs):
    x = np.ascontiguousarray(np.asarray(inputs["x"], np.float32))
    fwd = np.asarray(inputs["fwd_mem"], np.float32)
    rev = np.asarray(inputs["rev_mem"], np.float32)
    Wq = np.asarray(inputs["Wq"], np.float32)
    Wk = np.asarray(inputs["Wk"], np.float32)
    Wv = np.asarray(inputs["Wv"], np.float32)
    Wo = np.asarray(inputs["Wo"], np.float32)
    gate_w = np.asarray(inputs["gate_w"], np.float32)
    gate_b = np.asarray(inputs["gate_b"], np.float32)
    canon_w = np.asarray(inputs["canon_w"], np.float32)
    canon_bias = np.asarray(inputs["canon_bias"], np.float32)

    Wg = (gate_w.astype(np.float64) @ Wq.astype(np.float64)).astype(np.float32)

    ind = np.zeros((128, 256), np.float32)
    for base in (0, 64):
        ind[base + 0, 0:64] = 0.5       # g_A = .5*tanhA + .5*ones
        ind[base + 2, 0:64] = 0.5
        ind[base + 3, 64:128] = 1.0     # r_A = recipZ_A
        ind[base + 1, 128:192] = 0.5    # g_B
        ind[base + 2, 128:192] = 0.5
        ind[base + 4, 192:256] = 1.0    # r_B
    ind = _b16(ind)

    per_b, per_g = [], []
    for b in range(B):
        per_b.append({
            "xT": _b16(x[b].T),
            "memT": _b16(np.concatenate([fwd[b], rev[b]], axis=0).T),
        })
    for g in range(G):
        cs = slice(g * CPG, (g + 1) * CPG)
        WvTa = np.zeros((C, 65 * HPG), np.float32)
        for h in range(HPG):
            rows = Wv[g * CPG + h * HD: g * CPG + (h + 1) * HD]
            WvTa[:, 65 * h:65 * h + 64] = rows.T
        hs = slice(g * HPG, (g + 1) * HPG)
        per_g.append({
            "WqT": _b16(Wq[cs].T),
            "WkT": _b16(Wk[cs].T),
            "WvTa": _b16(WvTa),
            "WgT": _b16(Wg[hs].T),
            "gb2": np.ascontiguousarray(gate_b[hs] * 0.5).reshape(HPG, 1),
            "WoT": _b16(Wo[:, cs].T),
            "ind": ind,
            "cw": np.ascontiguousarray(canon_w[cs, 0, :]),
            "cb": np.ascontiguousarray(canon_bias[cs]).reshape(CPG, 1),
        })
    return per_b, per_g


LAST_EXEC_NS = None
LAST_RESULTS = None


def kernel(**inputs):
    global LAST_EXEC_NS, LAST_RESULTS
    from concourse.bass_utils import run_bass_kernel_spmd

    nc = _get_program()
    per_b, per_g = host_prep(inputs)
    in_maps = []
    for core in range(8):
        b, g = divmod(core, G)
        m = {}
        m.update(per_b[b])
        m.update(per_g[g])
        in_maps.append(m)

    trace = bool(int(os.environ.get("KERNEL_TRACE", "0")))
    kw = {}
    if trace:
        tcores = os.environ.get("KERNEL_TRACE_CORES", "0")
        kw = dict(
            trace=True,
            trace_cores=[int(c) for c in tcores.split(",")],
            tmpdir=os.environ.get("KERNEL_TRACE_DIR", None),
        )
    res = run_bass_kernel_spmd(nc, in_maps, core_ids=list(range(8)), **kw)
    LAST_EXEC_NS = res.exec_time_ns
    LAST_RESULTS = res
    outp = np.zeros((B, T, C), np.float32)
    for core in range(8):
        b = core // G
        outp[b] += np.asarray(res.results[core]["out"], np.float32)
    return outp


# revision 12
# speedup vs baseline: 1.5994x; 1.2526x over previous
"""Trainium2 Bass kernel for nn_CMAModel (memory-augmented causal attention).

Sharding: 8 cores = 2 batches x 4 head-groups. Each core handles one batch and
4 heads (256 channels); the output projection is row-parallel and the 4
per-batch partials are summed on the host (bf16 partials).

Per-core device program (channels on partitions, bf16 matmul operands):
  proj: qT/kT (paired PSUM banks), V rows (with ones col for row-sums),
        gate logits -> tanh (same ACT table set as Exp; sigmoid = .5*tanh+.5)
  attention per (head-pair mq, T-chunk j): for each 128-row S-tile,
        scoresT for heads A,B as two K=64 row-tiled matmuls into a 2-bank
        PSUM pair -> ONE Exp activation over both banks -> causal tri-mask
        on diagonal tiles -> PV matmuls accumulate Ac/Am [65,512] per head
        (ones col gives row-sums Z for free)
  combine: Y = (Ac + g*Am)/Z. g and 1/Z are partition-broadcast with rank-1
        matmuls on the PE (lhsT=indicator const, rhs=rows of gzt), so no
        DRAM round trips. Z-recip runs 128-wide via small SBUF reshape DMAs.
  conv + out-proj interleaved per T-chunk: depthwise causal conv K=4 +
        residual + bias, then out partial [512,1024] -> DRAM (bf16).
"""
import contextlib
import ctypes
import os
import sys
import types

import numpy as np

# ---------------------------------------------------------------- constants
B, T, C = 2, 2048, 1024
H, HD = 16, 64
M = 256
G = 4                 # head-groups (cores per batch)
HPG = H // G          # 4 heads per core
CPG = HPG * HD        # 256 channels per core
S = T + 2 * M         # 2560 kv rows
SM = 2 * M            # 512 memory rows
NKT = C // 128        # 8 contraction tiles
NST = S // 128        # 20 S tiles (16 chunk + 4 mem)
TC = 512              # T chunk size
NTC = T // TC         # 4
K = 4                 # conv taps
SCALE = 1.0 / float(np.sqrt(HD))

_BUILT = None


# ------------------------------------------------------- axon NTFF hook shim
def _install_ntff_hook():
    """The agent image lacks antenv.axon_hooks; synthesize it so
    run_bass_kernel_spmd(trace=True) can capture NTFF profiles."""
    if "antenv.axon_hooks" in sys.modules:
        return
    so_path = "/opt/axon/libaxon_pjrt.so"
    hook = None
    if os.path.exists(so_path):
        try:
            lib = ctypes.CDLL(so_path)
            if hasattr(lib, "axon_start_nrt_profile"):
                lib.axon_start_nrt_profile.argtypes = [
                    ctypes.POINTER(ctypes.c_int64),
                    ctypes.c_size_t,
                ]
                lib.axon_start_nrt_profile.restype = ctypes.c_int64
                lib.axon_stop_nrt_profile.argtypes = [ctypes.c_char_p]
                lib.axon_stop_nrt_profile.restype = ctypes.c_int64

                @contextlib.contextmanager
                def _hook(output_dir, device_ids):
                    import jax

                    jax.devices()
                    if device_ids:
                        ids = (ctypes.c_int64 * len(device_ids))(*device_ids)
                        rc = lib.axon_start_nrt_profile(ids, len(device_ids))
                    else:
                        rc = lib.axon_start_nrt_profile(None, 0)
                    if rc != 0:
                        raise RuntimeError(f"axon_start_nrt_profile rc={rc}")
                    try:
                        yield
                    finally:
                        n = lib.axon_stop_nrt_profile(str(output_dir).encode())
                        if n < 0:
                            raise RuntimeError(f"axon_stop_nrt_profile rc={n}")

                hook = _hook
        except OSError:
            pass
    mod = types.ModuleType("antenv.axon_hooks")
    mod.get_axon_ntff_profile_hook = lambda: hook
    mod.set_axon_ntff_profile_hook = lambda h: None
    sys.modules["antenv.axon_hooks"] = mod


# ------------------------------------------------------------- device build
def _build_program():
    import concourse.tile as tile
    from concourse import bacc, mybir
    from concourse.masks import make_upper_triangular

    f32 = mybir.dt.float32
    mdt = mybir.dt.bfloat16

    nc = bacc.Bacc("TRN2", target_bir_lowering=False, debug=False, num_devices=8)

    xT = nc.dram_tensor("xT", [C, T], mdt, kind="ExternalInput").ap()
    memT = nc.dram_tensor("memT", [C, SM], mdt, kind="ExternalInput").ap()
    WqT = nc.dram_tensor("WqT", [C, CPG], mdt, kind="ExternalInput").ap()
    WkT = nc.dram_tensor("WkT", [C, CPG], mdt, kind="ExternalInput").ap()
    WvTa = nc.dram_tensor("WvTa", [C, 65 * HPG], mdt, kind="ExternalInput").ap()
    WgT = nc.dram_tensor("WgT", [C, HPG], mdt, kind="ExternalInput").ap()
    gb2 = nc.dram_tensor("gb2", [HPG, 1], f32, kind="ExternalInput").ap()
    WoT = nc.dram_tensor("WoT", [CPG, C], mdt, kind="ExternalInput").ap()
    ind = nc.dram_tensor("ind", [128, 256], mdt, kind="ExternalInput").ap()
    cw = nc.dram_tensor("cw", [CPG, K], f32, kind="ExternalInput").ap()
    cb = nc.dram_tensor("cb", [CPG, 1], f32, kind="ExternalInput").ap()
    out = nc.dram_tensor("out", [T, C], mdt, kind="ExternalOutput").ap()

    Exp = mybir.ActivationFunctionType.Exp
    Tanh = mybir.ActivationFunctionType.Tanh

    with tile.TileContext(nc) as tc:
        with contextlib.ExitStack() as ctx:
            const = ctx.enter_context(tc.tile_pool(name="const", bufs=1))
            xpool = ctx.enter_context(tc.tile_pool(name="xpool", bufs=2))
            sb = ctx.enter_context(tc.tile_pool(name="sb", bufs=1))
            work = ctx.enter_context(tc.tile_pool(name="work", bufs=4))
            small = ctx.enter_context(tc.tile_pool(name="small", bufs=1))
            psum = ctx.enter_context(
                tc.tile_pool(name="psum", bufs=1, space="PSUM")
            )

            # ---- constants / weights
            wq_s = const.tile([128, NKT, CPG], mdt)
            nc.sync.dma_start(out=wq_s, in_=WqT.rearrange("(a p) n -> p a n", p=128))
            wk_s = const.tile([128, NKT, CPG], mdt)
            nc.sync.dma_start(out=wk_s, in_=WkT.rearrange("(a p) n -> p a n", p=128))
            wva_s = const.tile([128, NKT, 65 * HPG], mdt)
            nc.sync.dma_start(out=wva_s, in_=WvTa.rearrange("(a p) n -> p a n", p=128))
            wg_s = const.tile([128, NKT, HPG], mdt)
            nc.sync.dma_start(out=wg_s, in_=WgT.rearrange("(a p) n -> p a n", p=128))
            wo_s = const.tile([128, 2, C], mdt)
            nc.sync.dma_start(out=wo_s, in_=WoT.rearrange("(a p) n -> p a n", p=128))
            cw_s = const.tile([128, 2, K], f32)
            nc.sync.dma_start(out=cw_s, in_=cw.rearrange("(a p) n -> p a n", p=128))
            cb_s = const.tile([128, 2, 1], f32)
            nc.sync.dma_start(out=cb_s, in_=cb.rearrange("(a p) n -> p a n", p=128))
            gb2_s = const.tile([HPG, 1], f32)
            nc.sync.dma_start(out=gb2_s, in_=gb2)
            ind_s = const.tile([128, 256], mdt)
            nc.sync.dma_start(out=ind_s, in_=ind)

            trif = const.tile([128, 128], f32)
            make_upper_triangular(nc, trif, val=1.0, diag=True)
            tri2 = const.tile([128, 2, 128], mdt)
            nc.vector.tensor_copy(tri2[:, 0, :], trif)
            nc.vector.tensor_copy(tri2[:, 1, :], trif)

            # ---- persistent activations
            # qkT_s[:, m, 0, t] = qT, [:, m, 1, t] = kT  (m = channel half)
            qkT_s = sb.tile([128, 2, 2, T], mdt)
            kTm_s = sb.tile([128, 2, SM], mdt)
            V_s = sb.tile([128, NST, 65 * HPG], mdt)
            # gzt rows (base b=64*mq): b+0 tanh_A, b+1 tanh_B, b+2 ones,
            # b+3 recipZ_A, b+4 recipZ_B
            gzt = sb.tile([128, T], mdt)
            # attnout[:, mq, 0, t] = Y, [:, mq, 1, t] = conv result
            attnout = sb.tile([128, 2, 2, T], mdt)

            # one-time inits: ones col in V, gzt ones + recip rows (rows 0-1 /
            # 64-65 are overwritten by the gate tanh per chunk; engine ops
            # need 32-aligned partition starts so memset the whole block)
            oc = V_s[:, :, 64:65 * HPG:65]
            nc.vector.memset(oc, 1.0)
            for base in (0, 64):
                nc.vector.memset(gzt[base:base + 5, :], 1.0)

            xTr = xT.rearrange("(a p) t -> p a t", p=128)

            def proj_chunk(xh, tglob, tloc, on_act):
                """q/k/V/gate projections for T columns [tglob, tglob+512).
                on_act: route PSUM->SBUF copies to ScalarE (idle early) or
                VectorE (when ScalarE is busy with attention exps)."""
                cp = nc.scalar.copy if on_act else nc.vector.tensor_copy
                for m in range(2):
                    qk = psum.tile([128, 2, TC], f32, tag="pp", bufs=2,
                                   name=f"qk{tglob}_{m}")
                    for w, ws in ((0, wq_s), (1, wk_s)):
                        for k in range(NKT):
                            nc.tensor.matmul(
                                qk[:, w, :],
                                ws[:, k, m * 128:(m + 1) * 128],
                                xh[:, k, tloc:tloc + TC],
                                start=(k == 0),
                                stop=(k == NKT - 1),
                            )
                    cp(qkT_s[:, m, :, tglob:tglob + TC], qk)
                for mt in range(TC // 128):
                    st = tglob // 128 + mt
                    pv = psum.tile([128, 65 * HPG], f32, tag="pp", bufs=2,
                                   name=f"pv{st}")
                    for k in range(NKT):
                        nc.tensor.matmul(
                            pv,
                            xh[:, k, tloc + mt * 128:tloc + (mt + 1) * 128],
                            wva_s[:, k, :],
                            start=(k == 0),
                            stop=(k == NKT - 1),
                        )
                    # copy the 4 x 64 v-blocks, skipping the ones columns
                    cp(
                        V_s[:, st, :].rearrange("p (h c) -> p h c", c=65)[:, :, 0:64],
                        pv.rearrange("p (h c) -> p h c", c=65)[:, :, 0:64],
                    )
                pg = psum.tile([HPG, TC], f32, tag="pp", bufs=2,
                               name=f"pg{tglob}")
                for k in range(NKT):
                    nc.tensor.matmul(
                        pg,
                        wg_s[:, k, :],
                        xh[:, k, tloc:tloc + TC],
                        start=(k == 0),
                        stop=(k == NKT - 1),
                    )
                # sigmoid(l+b) = .5*tanh((l+b)/2) + .5; the .5 affine folds
                # into the gbc broadcast matmul via the ind/ones-row coeffs
                gtmp = small.tile([HPG, TC], mdt, tag="gt", bufs=2,
                                  name=f"gt{tglob}")
                nc.scalar.activation(
                    gtmp, pg, Tanh, bias=gb2_s, scale=0.5,
                )
                nc.sync.dma_start(
                    out=gzt[0:2, tglob:tglob + TC], in_=gtmp[0:2, :]
                )
                nc.sync.dma_start(
                    out=gzt[64:66, tglob:tglob + TC], in_=gtmp[2:4, :]
                )

            def proj_mem(mems):
                mk = psum.tile([128, 2, SM], f32, tag="pp", bufs=2, name="mk")
                for m in range(2):
                    for k in range(NKT):
                        nc.tensor.matmul(
                            mk[:, m, :],
                            wk_s[:, k, m * 128:(m + 1) * 128],
                            mems[:, k, :],
                            start=(k == 0),
                            stop=(k == NKT - 1),
                        )
                nc.scalar.copy(kTm_s, mk)
                for mt in range(SM // 128):
                    st = 16 + mt
                    pv = psum.tile([128, 65 * HPG], f32, tag="pp", bufs=2,
                                   name=f"pvm{mt}")
                    for k in range(NKT):
                        nc.tensor.matmul(
                            pv,
                            mems[:, k, mt * 128:(mt + 1) * 128],
                            wva_s[:, k, :],
                            start=(k == 0),
                            stop=(k == NKT - 1),
                        )
                    nc.scalar.copy(
                        V_s[:, st, :].rearrange("p (h c) -> p h c", c=65)[:, :, 0:64],
                        pv.rearrange("p (h c) -> p h c", c=65)[:, :, 0:64],
                    )

            def attn_block(mq, j):
                """Attention accumulation for head pair (2mq, 2mq+1), chunk j.
                Emits everything up to uY = Ac + g*Am (which frees the PSUM
                accumulators without waiting for the Z-reciprocal chain) and
                returns a finish() closure — the reciprocal-dependent final
                multiply — to be emitted after the NEXT block's matmuls so
                the in-order PE queue never stalls on the Z chain."""
                base = 64 * mq
                hA, hB = 2 * mq, 2 * mq + 1
                nct = 4 * (j + 1)
                js = TC * j
                # gate broadcast first: depends only on proj tanh rows, and
                # the PSUM tile is released after one short SBUF copy
                gp = psum.tile([128, 2, TC], f32, tag="pp", bufs=2,
                               name=f"gp{mq}_{j}")
                for hb in range(2):
                    nc.tensor.matmul(
                        gp[0:64, hb, :],
                        ind_s[base:base + 5, 128 * hb:128 * hb + 64],
                        gzt[base:base + 5, js:js + TC],
                        start=True,
                        stop=True,
                    )
                gbS = small.tile([64, 2, TC], mdt, tag="gbS", bufs=3,
                                 name=f"gb{mq}_{j}")
                if mq == 0:
                    nc.scalar.copy(gbS, gp[0:64, :, :])
                else:
                    nc.vector.tensor_copy(gbS, gp[0:64, :, :])
                AcAm_A = psum.tile([128, 2, TC], f32, tag="pa", bufs=2,
                                   name=f"aa{mq}_{j}")
                AcAm_B = psum.tile([128, 2, TC], f32, tag="pa", bufs=2,
                                   name=f"ab{mq}_{j}")
                for i in range(nct + 4):
                    is_mem = i >= nct
                    si = (16 + i - nct) if is_mem else i
                    off = 0
                    if not is_mem and si >= 4 * j:
                        off = 128 * si - TC * j
                    sp = psum.tile([128, 2, TC], f32, tag="pp", bufs=2,
                                   name=f"sp{mq}_{j}_{i}")
                    for b, ro in ((0, 0), (1, 64)):
                        kt = (
                            qkT_s[ro:ro + 64, mq, 1, si * 128:(si + 1) * 128]
                            if si < 16
                            else kTm_s[ro:ro + 64, mq,
                                       (si - 16) * 128:(si - 15) * 128]
                        )
                        nc.tensor.matmul(
                            sp[:, b, off:],
                            kt,
                            qkT_s[ro:ro + 64, mq, 0, js + off:js + TC],
                            start=True,
                            stop=True,
                        )
                    Pt = work.tile([128, 2, TC], mdt, tag="P")
                    nc.scalar.activation(
                        Pt[:, :, off:], sp[:, :, off:], Exp, scale=SCALE
                    )
                    if not is_mem and si >= 4 * j:
                        nc.vector.tensor_mul(
                            Pt[:, :, off:off + 128], Pt[:, :, off:off + 128],
                            tri2,
                        )
                    cm = 1 if is_mem else 0
                    first = (i == 0) or (i == nct)
                    last = (i == nct - 1) or (i == nct + 3)
                    nc.tensor.matmul(
                        AcAm_A[0:65, cm, off:],
                        V_s[:, si, 65 * hA:65 * hA + 65],
                        Pt[:, 0, off:],
                        start=first,
                        stop=last,
                    )
                    nc.tensor.matmul(
                        AcAm_B[0:65, cm, off:],
                        V_s[:, si, 65 * hB:65 * hB + 65],
                        Pt[:, 1, off:],
                        start=first,
                        stop=last,
                    )
                # Z rows -> 128-wide reciprocal -> gzt recip rows (TT may
                # read at most one PSUM operand, so copy then add)
                zt = small.tile([128, 2, TC], f32, tag="zt", bufs=2,
                                name=f"zt{mq}_{j}")
                uYs = []
                for b, AcAm in ((0, AcAm_A), (1, AcAm_B)):
                    nc.vector.tensor_copy(zt[64:65, b, :], AcAm[64:65, 0, :])
                    nc.vector.tensor_add(
                        zt[64:65, b, :], zt[64:65, b, :], AcAm[64:65, 1, :]
                    )
                    # uY = Ac + g*Am consumes the accumulators now
                    uY = small.tile([64, TC], mdt, tag="uY", bufs=5,
                                    name=f"uY{mq}_{j}_{b}")
                    nc.vector.tensor_mul(uY, AcAm[0:64, 1, :], gbS[:, b, :])
                    nc.vector.tensor_add(uY, uY, AcAm[0:64, 0, :])
                    uYs.append(uY)
                zrf = small.tile([128, 8], f32, tag="zrf", bufs=2,
                                 name=f"zrf{mq}_{j}")
                nc.sync.dma_start(out=zrf, in_=zt[64:65, :, :])
                zrg = small.tile([128, 8], f32, tag="zrg", bufs=2,
                                 name=f"zrg{mq}_{j}")
                nc.vector.reciprocal(zrg, zrf)
                zrb = small.tile([128, 8], mdt, tag="zrb", bufs=2,
                                 name=f"zrb{mq}_{j}")
                nc.vector.tensor_copy(zrb, zrg)
                nc.sync.dma_start(
                    out=gzt[base + 3:base + 4, js:js + TC], in_=zrb[0:64, :]
                )
                nc.sync.dma_start(
                    out=gzt[base + 4:base + 5, js:js + TC], in_=zrb[64:128, :]
                )

                def finish():
                    rp = psum.tile([128, 2, TC], f32, tag="pp", bufs=2,
                                   name=f"rp{mq}_{j}")
                    for hb in range(2):
                        nc.tensor.matmul(
                            rp[0:64, hb, :],
                            ind_s[base:base + 5,
                                  64 + 128 * hb:128 + 128 * hb],
                            gzt[base:base + 5, js:js + TC],
                            start=True,
                            stop=True,
                        )
                    rbS = small.tile([64, 2, TC], mdt, tag="rbS", bufs=2,
                                     name=f"rb{mq}_{j}")
                    if mq == 0:
                        nc.vector.tensor_copy(rbS, rp[0:64, :, :])
                    else:
                        nc.scalar.copy(rbS, rp[0:64, :, :])
                    nc.vector.tensor_mul(
                        attnout[0:64, mq, 0, js:js + TC], uYs[0], rbS[:, 0, :]
                    )
                    ybt = small.tile([64, TC], mdt, tag="ybt", bufs=2,
                                     name=f"yb{mq}_{j}")
                    nc.vector.tensor_mul(ybt, uYs[1], rbS[:, 1, :])
                    nc.sync.dma_start(
                        out=attnout[64:128, mq, 0, js:js + TC], in_=ybt
                    )

                return finish

            def conv_chunk(j, mq):
                """depthwise causal conv + residual + bias on chunk j."""
                js, je = TC * j, TC * (j + 1)
                y = attnout[:, mq, 0, :]
                R = attnout[:, mq, 1, :]
                nc.vector.tensor_scalar_add(
                    R[:, js:je], y[:, js:je], cb_s[:, mq, :]
                )
                ctmp = small.tile([128, TC], mdt, tag="ctmp", bufs=2,
                                  name=f"ct{j}_{mq}")
                for k in range(K):
                    sh = K - 1 - k
                    if sh == 0:
                        nc.vector.tensor_scalar_mul(
                            ctmp, y[:, js:je], cw_s[:, mq, k:k + 1]
                        )
                        nc.vector.tensor_add(R[:, js:je], R[:, js:je], ctmp)
                    else:
                        a = sh if j == 0 else 0
                        nc.vector.tensor_scalar_mul(
                            ctmp[:, a:], y[:, js + a - sh:je - sh],
                            cw_s[:, mq, k:k + 1],
                        )
                        nc.vector.tensor_add(
                            R[:, js + a:je], R[:, js + a:je], ctmp[:, a:]
                        )

            def outproj_chunk(j):
                for mt in range(TC // 128):
                    row = j * 4 + mt
                    po = psum.tile([128, 2, TC], f32, tag="pp", bufs=2,
                                   name=f"po{row}")
                    for nb in range(2):
                        for p in range(2):
                            nc.tensor.matmul(
                                po[:, nb, :],
                                attnout[:, p, 1, row * 128:(row + 1) * 128],
                                wo_s[:, p, nb * TC:(nb + 1) * TC],
                                start=(p == 0),
                                stop=(p == 1),
                            )
                    ot = small.tile([128, 2, TC], mdt, tag="ot", bufs=3,
                                    name=f"ot{row}")
                    if mt % 2 == 0:
                        nc.vector.tensor_copy(ot, po)
                    else:
                        nc.scalar.copy(ot, po)
                    nc.sync.dma_start(
                        out=out[row * 128:(row + 1) * 128, :].rearrange(
                            "p (a n) -> p a n", a=2
                        ),
                        in_=ot,
                    )

            # ---- emission: proj c0, mem, c1, then attn j interleaved with
            # remaining proj chunks so PE always has dense work and ACT/DVE
            # overlap.
            xh0 = xpool.tile([128, NKT, T // 2], mdt, tag="xbig", name="xh0")
            for k in range(NKT):
                nc.sync.dma_start(out=xh0[:, k, :], in_=xTr[:, k, :T // 2])
            mems = xpool.tile([128, NKT, SM], mdt, tag="xmem", name="xmem")
            nc.sync.dma_start(out=mems, in_=memT.rearrange("(a p) t -> p a t", p=128))

            proj_chunk(xh0, 0, 0, on_act=True)
            proj_mem(mems)
            proj_chunk(xh0, TC, TC, on_act=True)

            xh1 = xpool.tile([128, NKT, T // 2], mdt, tag="xbig", name="xh1")
            for k in range(NKT):
                nc.sync.dma_start(out=xh1[:, k, :], in_=xTr[:, k, T // 2:])

            # Pipelined emission: each block's reciprocal-dependent finish()
            # lands after the next block's matmul burst; conv one slot later;
            # outproj one more. Keeps the in-order PE queue stall-free.
            pending = []

            def drain(now):
                pending.sort(key=lambda e: e[0])
                while pending and pending[0][0] <= now:
                    pending.pop(0)[1]()

            slot = 0
            for j in range(NTC):
                if j >= 2:
                    tg = j * TC
                    proj_chunk(xh1, tg, tg - 2 * TC, on_act=False)
                for mq in range(2):
                    fin = attn_block(mq, j)
                    drain(slot)
                    pending.append((slot + 1, fin))
                    slot += 1
                jj = j

                def conv_out(jc=jj):
                    for mq in range(2):
                        conv_chunk(jc, mq)

                pending.append((slot + 1, conv_out))
                pending.append((slot + 2, lambda jc=jj: outproj_chunk(jc)))
            drain(slot + 2)

    nc.compile()
    return nc


def _get_program():
    global _BUILT
    if _BUILT is None:
        _install_ntff_hook()
        _BUILT = _build_program()
    return _BUILT


# --------------------------------------------------------------- host side
def _b16(a):
    import ml_dtypes

    return np.ascontiguousarray(a, np.float32).astype(ml_dtypes.bfloat16)


def host_prep(inputs):
    x = np.ascontiguousarray(np.asarray(inputs["x"], np.float32))
    fwd = np.asarray(inputs["fwd_mem"], np.float32)
    rev = np.asarray(inputs["rev_mem"], np.float32)
    Wq = np.asarray(inputs["Wq"], np.float32)
    Wk = np.asarray(inputs["Wk"], np.float32)
    Wv = np.asarray(inputs["Wv"], np.float32)
    Wo = np.asarray(inputs["Wo"], np.float32)
    gate_w = np.asarray(inputs["gate_w"], np.float32)
    gate_b = np.asarray(inputs["gate_b"], np.float32)
    canon_w = np.asarray(inputs["canon_w"], np.float32)
    canon_bias = np.asarray(inputs["canon_bias"], np.float32)

    Wg = (gate_w.astype(np.float64) @ Wq.astype(np.float64)).astype(np.float32)

    ind = np.zeros((128, 256), np.float32)
    for base in (0, 64):
        ind[base + 0, 0:64] = 0.5       # g_A = .5*tanhA + .5*ones
        ind[base + 2, 0:64] = 0.5
        ind[base + 3, 64:128] = 1.0     # r_A = recipZ_A
        ind[base + 1, 128:192] = 0.5    # g_B
        ind[base + 2, 128:192] = 0.5
        ind[base + 4, 192:256] = 1.0    # r_B
    ind = _b16(ind)

    per_b, per_g = [], []
    for b in range(B):
        per_b.append({
            "xT": _b16(x[b].T),
            "memT": _b16(np.concatenate([fwd[b], rev[b]], axis=0).T),
        })
    for g in range(G):
        cs = slice(g * CPG, (g + 1) * CPG)
        WvTa = np.zeros((C, 65 * HPG), np.float32)
        for h in range(HPG):
            rows = Wv[g * CPG + h * HD: g * CPG + (h + 1) * HD]
            WvTa[:, 65 * h:65 * h + 64] = rows.T
        hs = slice(g * HPG, (g + 1) * HPG)
        per_g.append({
            "WqT": _b16(Wq[cs].T),
            "WkT": _b16(Wk[cs].T),
            "WvTa": _b16(WvTa),
            "WgT": _b16(Wg[hs].T),
            "gb2": np.ascontiguousarray(gate_b[hs] * 0.5).reshape(HPG, 1),
            "WoT": _b16(Wo[:, cs].T),
            "ind": ind,
            "cw": np.ascontiguousarray(canon_w[cs, 0, :]),
            "cb": np.ascontiguousarray(canon_bias[cs]).reshape(CPG, 1),
        })
    return per_b, per_g


LAST_EXEC_NS = None
LAST_RESULTS = None


def kernel(**inputs):
    global LAST_EXEC_NS, LAST_RESULTS
    from concourse.bass_utils import run_bass_kernel_spmd

    nc = _get_program()
    per_b, per_g = host_prep(inputs)
    in_maps = []
    for core in range(8):
        b, g = divmod(core, G)
        m = {}
        m.update(per_b[b])
        m.update(per_g[g])
        in_maps.append(m)

    trace = bool(int(os.environ.get("KERNEL_TRACE", "0")))
    kw = {}
    if trace:
        tcores = os.environ.get("KERNEL_TRACE_CORES", "0")
        kw = dict(
            trace=True,
            trace_cores=[int(c) for c in tcores.split(",")],
            tmpdir=os.environ.get("KERNEL_TRACE_DIR", None),
        )
    res = run_bass_kernel_spmd(nc, in_maps, core_ids=list(range(8)), **kw)
    LAST_EXEC_NS = res.exec_time_ns
    LAST_RESULTS = res
    outp = np.zeros((B, T, C), np.float32)
    for core in range(8):
        b = core // G
        outp[b] += np.asarray(res.results[core]["out"], np.float32)
    return outp
